# revision 55
# baseline (speedup 1.0000x reference)
"""Trainium2 Bass kernel for nn_CoarseGrainUpdate (gnn_message_passing).

Two launches on 8 edge-sharded cores (this runtime has no working dynamic
gather DMA, so all indexed gathers are host-side data marshaling; every
arithmetic op runs on device):

  Launch A: scatter-mean numerator/denominator as a fixed-width padded
            segment reduction (Pool windowed reduce), dst-range sharded.
            max(cnt,1) + divide on device.
  Launch B: edge-sharded streaming feature compute in component-PLANAR
            layout ([128, plane, col]): vec (DVE), |v|^2 (Pool),
            sqrt (ACT), 1/d (DVE), the whole 16-plane RBF chain on ACT
            (per-plane Square(d - mu_m) with bias APs, one Exp), and the
            spherical harmonics as bf16 DVE products. The [3,E,25]
            output is written as bf16 (absmax-relative rounding ~1e-2
            vs the 2e-2 gate), halving the dominant output HBM traffic.

Both builders take reps=None (real kernel, external IO) or reps=K
(benchmark variant: identical per-iteration instruction stream wrapped
in a hardware For_i loop, big IO on Internal DRAM). test.py uses the
bench variants for differential wall timing: (wall[K] - wall[1])/(K-1)
cancels the axon tunnel/dispatch overhead and leaves HW exec time.
"""
import numpy as np
import concourse.bass as bass
import concourse.bacc as bacc
import concourse.tile as tile
import concourse.mybir as mybir
import concourse.bass_utils as bass_utils

N_CORES = 8
N_FRAME = 100000
N_TFN = 25000
E = 2000000
EPC = E // N_CORES            # 250000 edges per core
NUM_RBF = 16
EPS = 1e-8
SIGMA = 1.25                  # (20-0)/16
MU = np.linspace(0.0, 20.0, NUM_RBF, dtype=np.float32)
S3 = 1.7320508075688772
S5 = 2.23606797749979
S15 = 3.872983346207417

# launch B tiling: 3 edge types x NCH chunks of [128, CB] edges
CB = 489
NCH = 4
PADE = 128 * CB * NCH         # 250368 >= EPC
# RBF plane split across engines. HW A/B sweeps (kd/kp in {0,5}x{0,5}):
# all-ACT (0,0) runs 272us/launch vs 330-375us for any DVE/Pool offload —
# the broadcast w=d-mu tensor_tensors plus the cross-engine u handoff cost
# far more on real HW than the cost model predicts. Keep the whole RBF
# chain (Square(d - mu_m) then Exp) ACT-local.
K_D = 0                       # RBF planes built on DVE
K_P = 0                       # RBF planes built on Pool (ACT does the rest)

# launch A segment tiling (dst-range shard: core k owns segs [k*3200,(k+1)*3200))
SEG_PAD = 25600
SEG_PER_CORE = SEG_PAD // N_CORES   # 3200
SEG_PER_PART = SEG_PER_CORE // 128  # 25

f32 = mybir.dt.float32
bf16 = mybir.dt.bfloat16

_cache = {}


def _build_launch_a(W, reps=None, cb=None):
    nc = bacc.Bacc("TRN2", target_bir_lowering=False, debug=False,
                   num_devices=N_CORES)
    P25 = SEG_PER_PART
    FW = P25 * W
    big_kind = "Internal" if reps else "ExternalInput"
    grid_d = nc.dram_tensor("grid", [128, 4, FW], f32, kind=big_kind)
    out_d = nc.dram_tensor("tfn", [128, 3 * P25], f32, kind="ExternalOutput")
    if reps:
        tick_d = nc.dram_tensor("tick", [128, 1], f32, kind="ExternalInput")
    add = mybir.AluOpType.add
    mul = mybir.AluOpType.mult
    with tile.TileContext(nc) as tc:
        with (tc.tile_pool(name="ga", bufs=3) as pool,
              tc.tile_pool(name="gb", bufs=1) as spool):
            red = spool.tile([128, 4, P25], f32, tag="red")
            rec = spool.tile([128, P25], f32, tag="rec")
            o = spool.tile([128, 3, P25], f32, tag="o")
            if reps:
                tick_t = spool.tile([128, 1], f32, tag="tick")
                nc.sync.dma_start(out=tick_t[:], in_=tick_d.ap())

            def body():
                for s0 in range(0, P25, 5):
                    g = pool.tile([128, 4, 5 * W], f32, tag="g")
                    nc.sync.dma_start(
                        out=g[:], in_=grid_d.ap()[:, :, s0 * W:(s0 + 5) * W])
                    nc.vector.tensor_reduce(
                        red[:, :, s0:s0 + 5],
                        g[:].rearrange("p c (s w) -> p c s w", w=W),
                        axis=mybir.AxisListType.X, op=add)
                nc.vector.tensor_scalar_max(rec[:], red[:, 3, :], 1.0)
                nc.vector.reciprocal(rec[:], rec[:])
                nc.vector.tensor_tensor(
                    out=o[:], in0=red[:, 0:3, :],
                    in1=rec[:].rearrange("p (o s) -> p o s", o=1)
                        .to_broadcast([128, 3, P25]),
                    op=mul)
                nc.sync.dma_start(out=out_d.ap(),
                                  in_=o[:].rearrange("p c s -> p (c s)"))

            if reps:
                with tc.For_i(0, reps):
                    body()
                # keep the ExternalInput live so the NEFF has >=1 input
                nc.vector.tensor_tensor(out=tick_t[:], in0=tick_t[:],
                                        in1=tick_t[:], op=add)
                nc.sync.dma_start(out=out_d.ap()[:, 0:1], in_=tick_t[:])
            else:
                body()
    nc.compile()
    return nc


def _build_launch_b(reps=None, cb=CB, nch=NCH, diag=None, kd=K_D, kp=K_P):
    """Planar edge-feature kernel.

    Per chunk [128, cb] edges: vd2 = DMA in, v=a-b (DVE), d2 =
    max(|v|^2, tiny) (Pool); mid = sqrt (ACT), 1/d (DVE); back = RBF
    w_m^2 fused per plane into ACT Square(d + (-mu_m)), one
    exp(-w^2/sigma^2) over all 16 planes (ACT), SH products (DVE bf16),
    DMA out bf16. The body is modulo-software-pipelined (DMA-in leads
    by 3 chunks, v/d2 by 2) with the sqrts batched 3 chunks at a time
    so ACT pays only 2 activation-table loads (sqrt set vs
    square/exp set, 1283ns each) per 3 chunks.
    """
    nc = bacc.Bacc("TRN2", target_bir_lowering=False, debug=False,
                   num_devices=N_CORES)
    big_kind_i = "Internal" if reps else "ExternalInput"
    big_kind_o = "Internal" if reps else "ExternalOutput"
    ab_d = nc.dram_tensor("ab", [3, nch, 128, 6, cb], f32, kind=big_kind_i)
    od_d = nc.dram_tensor("od", [3, nch, 128, 25 * cb], bf16, kind=big_kind_o)
    mu_d = nc.dram_tensor("negmu", [128, NUM_RBF], f32, kind="ExternalInput")
    if reps:
        tok_d = nc.dram_tensor("tok", [128, NUM_RBF], f32,
                               kind="ExternalOutput")
    sub = mybir.AluOpType.subtract
    mul = mybir.AluOpType.mult
    add = mybir.AluOpType.add
    SQ = mybir.ActivationFunctionType.Square
    SQRT = mybir.ActivationFunctionType.Sqrt
    EXP = mybir.ActivationFunctionType.Exp
    KDP = kd + kp
    nchunks = 3 * nch
    big = cb >= 600
    buf_ab = 3 if big else 4
    buf_v = 3 if big else 4
    buf_d2 = 3 if big else 4
    buf_di = 4 if big else 5
    buf_u = 1 if big else 2
    with tile.TileContext(nc) as tc:
        with (tc.tile_pool(name="io", bufs=2) as iop,
              tc.tile_pool(name="wk", bufs=2) as wkp):
            V, A, P = nc.vector, nc.scalar, nc.gpsimd
            negmu = iop.tile([128, NUM_RBF], f32, tag="negmu", bufs=1)
            nc.sync.dma_start(out=negmu[:], in_=mu_d.ap())
            if reps:
                tok_t = iop.tile([128, NUM_RBF], f32, tag="tok", bufs=1)
                V.tensor_tensor(out=tok_t[:], in0=negmu[:], in1=negmu[:],
                                op=add)
                nc.sync.dma_start(out=tok_d.ap(), in_=tok_t[:])
            # two rotating output buffers; the constant l0=1 plane (16) is
            # written once here and never touched again
            obufs = [iop.tile([128, 25, cb], bf16, tag="ob", name=f"ob{i}")
                     for i in range(2)]
            for ob in obufs:
                V.memset(ob[:, 16, :], 1.0)

            abufs = []
            if diag == "nodma":
                abufs = [iop.tile([128, 6, cb], f32, tag="ab", bufs=buf_ab,
                                  name=f"abf{i}") for i in range(buf_ab)]
                for abf in abufs:
                    V.memset(abf[:], 1.0)

            def dma_in(ci):
                if diag == "nodma":
                    return abufs[ci % buf_ab]
                t, n = divmod(ci, nch)
                ab = iop.tile([128, 6, cb], f32, tag="ab", bufs=buf_ab)
                nc.sync.dma_start(out=ab[:], in_=ab_d.ap()[t, n])
                return ab

            def vd2(ab, ci):
                # v = a-b (DVE), d2 = max(|v|^2, tiny) (Pool)
                v = wkp.tile([128, 3, cb], f32, tag="v", bufs=buf_v)
                d2 = wkp.tile([128, cb], f32, tag="d2", bufs=buf_d2)
                tq = wkp.tile([128, cb], f32, tag="tq", bufs=2)
                V.tensor_tensor(out=v[:], in0=ab[:, 0:3, :],
                                in1=ab[:, 3:6, :], op=sub)
                P.tensor_tensor(out=d2[:], in0=v[:, 0, :], in1=v[:, 0, :],
                                op=mul)
                P.tensor_tensor(out=tq[:], in0=v[:, 1, :], in1=v[:, 1, :],
                                op=mul)
                P.tensor_tensor(out=d2[:], in0=d2[:], in1=tq[:], op=add)
                P.tensor_tensor(out=tq[:], in0=v[:, 2, :], in1=v[:, 2, :],
                                op=mul)
                P.tensor_tensor(out=d2[:], in0=d2[:], in1=tq[:], op=add)
                P.tensor_scalar_max(d2[:], d2[:], 3e-16)
                return {"v": v, "d2": d2, "ci": ci}

            def mid(st):
                # d = sqrt(d2) (ACT, sqrt table set), 1/d (DVE)
                dp = wkp.tile([128, cb], f32, tag="dp", bufs=buf_di)
                inv = wkp.tile([128, cb], f32, tag="inv", bufs=buf_di)
                A.activation(dp[:], st["d2"][:], SQRT)
                V.reciprocal(inv[:], dp[:])
                st["dp"] = dp
                st["inv"] = inv

            def back(st):
                ci = st["ci"]
                t, n = divmod(ci, nch)
                v, dp, inv = st["v"], st["dp"], st["inv"]
                ob = obufs[ci % 2]
                u = wkp.tile([128, NUM_RBF, cb], bf16, tag="u", bufs=buf_u)
                # RBF: w_m^2 into the u staging tile (decouples RBF compute
                # from the 2-deep output rotation — writing ob directly
                # measured 30us/launch slower), then one exp into ob.
                # Planes [0:kd] via DVE / [kd:KDP] via Pool immediate-constant
                # tensor_scalar (broadcast-AP tensor_tensor forms of this
                # were 100us+ slower on HW), the rest fused on ACT via
                # Square(d + (-mu_m)).
                for m in range(kd):
                    V.tensor_scalar_add(u[:, m, :], dp[:], float(-MU[m]))
                for m in range(kd, KDP):
                    P.tensor_scalar_add(u[:, m, :], dp[:], float(-MU[m]))
                if KDP:
                    V.tensor_tensor(out=u[:, 0:KDP, :], in0=u[:, 0:KDP, :],
                                    in1=u[:, 0:KDP, :], op=mul)
                for m in range(KDP, NUM_RBF):
                    A.activation(u[:, m, :], dp[:], SQ,
                                 bias=negmu[:, m:m + 1])
                A.activation(ob[:, 0:NUM_RBF, :], u[:], EXP,
                             scale=-1.0 / (SIGMA * SIGMA))
                # SH block (bf16)
                r = wkp.tile([128, 3, cb], bf16, tag="r", bufs=1)
                rs = wkp.tile([128, 3, cb], bf16, tag="rs", bufs=1)
                z2 = wkp.tile([128, cb], bf16, tag="z2", bufs=1)
                m12 = wkp.tile([128, 2, cb], bf16, tag="m12", bufs=1)
                V.tensor_tensor(
                    out=r[:], in0=v[:],
                    in1=inv[:].rearrange("p (o c) -> p o c", o=1)
                        .to_broadcast([128, 3, cb]),
                    op=mul)
                V.tensor_scalar_mul(rs[:], r[:], S15)
                V.tensor_scalar_mul(ob[:, 17:20, :], r[:], S3)
                V.tensor_tensor(out=ob[:, 20:22, :], in0=rs[:, 0:2, :],
                                in1=r[:, 1:3, :], op=mul)
                V.tensor_tensor(out=ob[:, 23, :], in0=rs[:, 0, :],
                                in1=r[:, 2, :], op=mul)
                V.tensor_tensor(out=z2[:], in0=rs[:, 2, :], in1=r[:, 2, :],
                                op=mul)
                V.tensor_scalar(ob[:, 22, :], z2[:], 0.8660254037844386,
                                -0.5 * S5, op0=mul, op1=add)
                V.tensor_tensor(out=m12[:], in0=rs[:, 0:2, :],
                                in1=r[:, 0:2, :], op=mul)
                V.tensor_tensor(out=m12[:, 0, :], in0=m12[:, 0, :],
                                in1=m12[:, 1, :], op=sub)
                V.tensor_scalar(ob[:, 24, :], m12[:, 0, :], 0.5, None,
                                op0=mul)
                if diag != "nodma":
                    nc.sync.dma_start(out=od_d.ap()[t, n],
                                      in_=ob[:].rearrange("p f c -> p (f c)"))

            def body():
                # modulo schedule: DMA-in leads by 3 chunks, v/d2 by 2,
                # sqrt+recip batched per 3 chunks (2 ACT table loads per
                # batch), back at offset 0
                abs_ = {}
                sts = {}
                for ci in range(3):
                    abs_[ci] = dma_in(ci)
                for ci in range(2):
                    sts[ci] = vd2(abs_.pop(ci), ci)
                for c in range(nchunks):
                    if c % 3 == 0:
                        for ci in range(c, min(c + 3, nchunks)):
                            if ci not in sts:
                                sts[ci] = vd2(abs_.pop(ci), ci)
                            mid(sts[ci])
                    if c + 3 < nchunks:
                        abs_[c + 3] = dma_in(c + 3)
                    if c + 2 < nchunks and (c + 2) not in sts:
                        sts[c + 2] = vd2(abs_.pop(c + 2), c + 2)
                    back(sts.pop(c))

            if reps:
                with tc.For_i(0, reps):
                    body()
            else:
                body()
    nc.compile()
    return nc


def _seg_grids(trans_g, f_src, t_dst, W):
    """Host marshaling: place trans[f_src] rows + mask into padded per-core
    channel-planar segment grids [N_CORES, 128, 4, SEG_PER_PART*W]."""
    n = f_src.shape[0]
    order = np.argsort(t_dst, kind="stable")
    sd = t_dst[order]
    sf = f_src[order]
    starts = np.searchsorted(sd, np.arange(N_TFN))
    rank = np.arange(n) - starts[sd]
    core = sd // SEG_PER_CORE
    local = sd % SEG_PER_CORE
    p = local // SEG_PER_PART
    j = local % SEG_PER_PART
    FW = SEG_PER_PART * W
    grids = np.zeros((N_CORES, 128, 4, FW), np.float32)
    vals = trans_g[sf]  # [n, 3]
    pos = j * W + rank
    grids[core, p, 0, pos] = vals[:, 0]
    grids[core, p, 1, pos] = vals[:, 1]
    grids[core, p, 2, pos] = vals[:, 2]
    grids[core, p, 3, pos] = 1.0
    return grids


def _negmu_grid():
    return np.broadcast_to((-MU)[None, :],
                           (128, NUM_RBF)).astype(np.float32).copy()


def _marshal_b(trans, tfn_x, f_src, t_dst, t2t, t2f):
    """Planar per-core launch-B input grids [N_CORES, 3, NCH, 128, 6, CB].
    Pure index marshaling (gather + layout), no arithmetic."""
    ab = np.zeros((N_CORES, 3, NCH, 128, 6, CB), np.float32)
    pairs = ((trans[f_src], tfn_x[t_dst]),
             (tfn_x[t2t[0]], tfn_x[t2t[1]]),
             (tfn_x[t2f[0]], trans[t2f[1]]))
    tmp = np.zeros((PADE, 6), np.float32)
    for t, (arows, brows) in enumerate(pairs):
        for k in range(N_CORES):
            tmp[:EPC, 0:3] = arows[k * EPC:(k + 1) * EPC]
            tmp[:EPC, 3:6] = brows[k * EPC:(k + 1) * EPC]
            ab[k, t] = tmp.reshape(NCH, 128, CB, 6).transpose(0, 1, 3, 2)
    return ab


def _run_launch_a(trans, f_src, t_dst):
    cnts = np.bincount(t_dst, minlength=N_TFN)
    W = int(cnts.max())
    key = ("A", W)
    if key not in _cache:
        _cache[key] = _build_launch_a(W)
    ncA = _cache[key]
    grids = _seg_grids(trans, f_src, t_dst, W)
    in_maps = [{"grid": grids[k]} for k in range(N_CORES)]
    resA = bass_utils.run_bass_kernel_spmd(ncA, in_maps,
                                           core_ids=list(range(N_CORES)))
    tfn_x = np.zeros((SEG_PAD, 3), np.float32)
    for k in range(N_CORES):
        o = resA.results[k]["tfn"].reshape(128, 3, SEG_PER_PART)
        segs = (np.arange(128)[:, None] * SEG_PER_PART
                + np.arange(SEG_PER_PART)[None, :] + k * SEG_PER_CORE)
        tfn_x[segs.ravel()] = o.transpose(0, 2, 1).reshape(-1, 3)
    return tfn_x[:N_TFN]


def _run_launch_b(ab):
    if "B" not in _cache:
        _cache["B"] = _build_launch_b()
    ncB = _cache["B"]
    negmu = _negmu_grid()
    in_maps = [{"ab": ab[k], "negmu": negmu} for k in range(N_CORES)]
    resB = bass_utils.run_bass_kernel_spmd(ncB, in_maps,
                                           core_ids=list(range(N_CORES)))
    out = np.empty((3, E, NUM_RBF + 9), np.float32)
    for k in range(N_CORES):
        o = np.asarray(resB.results[k]["od"])
        o = o.reshape(3, NCH, 128, 25, CB).astype(np.float32)
        o = o.transpose(0, 1, 2, 4, 3).reshape(3, PADE, 25)
        out[:, k * EPC:(k + 1) * EPC, :] = o[:, :EPC, :]
    return out


def kernel(trans, frame2tfn_edge_index, tfn2tfn_edge_index,
           tfn2frame_edge_index, n_tfn):
    trans = np.asarray(trans, np.float32)
    f2t = np.asarray(frame2tfn_edge_index, np.int64)
    t2t = np.asarray(tfn2tfn_edge_index, np.int64)
    t2f = np.asarray(tfn2frame_edge_index, np.int64)
    f_src, t_dst = f2t[0], f2t[1]

    tfn_x = _run_launch_a(trans, f_src, t_dst)
    ab = _marshal_b(trans, tfn_x, f_src, t_dst, t2t, t2f)
    return _run_launch_b(ab)


# revision 59
# speedup vs baseline: 1.0894x; 1.0894x over previous
"""Trainium2 Bass kernel for nn_CoarseGrainUpdate (gnn_message_passing).

Two launches on 8 edge-sharded cores (this runtime has no working dynamic
gather DMA, so all indexed gathers are host-side data marshaling; every
arithmetic op runs on device):

  Launch A: scatter-mean numerator/denominator as a fixed-width padded
            segment reduction (Pool windowed reduce), dst-range sharded.
            max(cnt,1) + divide on device.
  Launch B: edge-sharded streaming feature compute in component-PLANAR
            layout ([128, plane, col]): vec (DVE), |v|^2 (Pool),
            sqrt (ACT), 1/d (DVE), the whole 16-plane RBF chain on ACT
            (per-plane Square(d - mu_m) with bias APs, one Exp), and the
            spherical harmonics as bf16 DVE products. The [3,E,25]
            output is written as bf16 (absmax-relative rounding ~1e-2
            vs the 2e-2 gate), halving the dominant output HBM traffic.

Both builders take reps=None (real kernel, external IO) or reps=K
(benchmark variant: identical per-iteration instruction stream wrapped
in a hardware For_i loop, big IO on Internal DRAM). test.py uses the
bench variants for differential wall timing: (wall[K] - wall[1])/(K-1)
cancels the axon tunnel/dispatch overhead and leaves HW exec time.
"""
import numpy as np
import concourse.bass as bass
import concourse.bacc as bacc
import concourse.tile as tile
import concourse.mybir as mybir
import concourse.bass_utils as bass_utils

N_CORES = 8
N_FRAME = 100000
N_TFN = 25000
E = 2000000
EPC = E // N_CORES            # 250000 edges per core
NUM_RBF = 16
EPS = 1e-8
SIGMA = 1.25                  # (20-0)/16
MU = np.linspace(0.0, 20.0, NUM_RBF, dtype=np.float32)
S3 = 1.7320508075688772
S5 = 2.23606797749979
S15 = 3.872983346207417

# launch B tiling: 3 edge types x NCH chunks of [128, CB] edges
CB = 489
NCH = 4
PADE = 128 * CB * NCH         # 250368 >= EPC
# RBF plane split across engines. HW A/B sweeps (kd/kp in {0,5}x{0,5}):
# all-ACT (0,0) runs 272us/launch vs 330-375us for any DVE/Pool offload —
# the broadcast w=d-mu tensor_tensors plus the cross-engine u handoff cost
# far more on real HW than the cost model predicts. Keep the whole RBF
# chain (Square(d - mu_m) then Exp) ACT-local.
K_D = 0                       # RBF planes built on DVE
K_P = 0                       # RBF planes built on Pool (ACT does the rest)

# launch A segment tiling (dst-range shard: core k owns segs [k*3200,(k+1)*3200))
SEG_PAD = 25600
SEG_PER_CORE = SEG_PAD // N_CORES   # 3200
SEG_PER_PART = SEG_PER_CORE // 128  # 25

f32 = mybir.dt.float32
bf16 = mybir.dt.bfloat16

_cache = {}


def _build_launch_a(W, reps=None, cb=None):
    nc = bacc.Bacc("TRN2", target_bir_lowering=False, debug=False,
                   num_devices=N_CORES)
    P25 = SEG_PER_PART
    FW = P25 * W
    big_kind = "Internal" if reps else "ExternalInput"
    grid_d = nc.dram_tensor("grid", [128, 4, FW], f32, kind=big_kind)
    out_d = nc.dram_tensor("tfn", [128, 3 * P25], f32, kind="ExternalOutput")
    if reps:
        tick_d = nc.dram_tensor("tick", [128, 1], f32, kind="ExternalInput")
    add = mybir.AluOpType.add
    mul = mybir.AluOpType.mult
    with tile.TileContext(nc) as tc:
        with (tc.tile_pool(name="ga", bufs=3) as pool,
              tc.tile_pool(name="gb", bufs=1) as spool):
            red = spool.tile([128, 4, P25], f32, tag="red")
            rec = spool.tile([128, P25], f32, tag="rec")
            o = spool.tile([128, 3, P25], f32, tag="o")
            if reps:
                tick_t = spool.tile([128, 1], f32, tag="tick")
                nc.sync.dma_start(out=tick_t[:], in_=tick_d.ap())

            def body():
                for s0 in range(0, P25, 5):
                    g = pool.tile([128, 4, 5 * W], f32, tag="g")
                    nc.sync.dma_start(
                        out=g[:], in_=grid_d.ap()[:, :, s0 * W:(s0 + 5) * W])
                    nc.vector.tensor_reduce(
                        red[:, :, s0:s0 + 5],
                        g[:].rearrange("p c (s w) -> p c s w", w=W),
                        axis=mybir.AxisListType.X, op=add)
                nc.vector.tensor_scalar_max(rec[:], red[:, 3, :], 1.0)
                nc.vector.reciprocal(rec[:], rec[:])
                nc.vector.tensor_tensor(
                    out=o[:], in0=red[:, 0:3, :],
                    in1=rec[:].rearrange("p (o s) -> p o s", o=1)
                        .to_broadcast([128, 3, P25]),
                    op=mul)
                nc.sync.dma_start(out=out_d.ap(),
                                  in_=o[:].rearrange("p c s -> p (c s)"))

            if reps:
                with tc.For_i(0, reps):
                    body()
                # keep the ExternalInput live so the NEFF has >=1 input
                nc.vector.tensor_tensor(out=tick_t[:], in0=tick_t[:],
                                        in1=tick_t[:], op=add)
                nc.sync.dma_start(out=out_d.ap()[:, 0:1], in_=tick_t[:])
            else:
                body()
    nc.compile()
    return nc


def _build_launch_b(reps=None, cb=CB, nch=NCH, diag=None, kd=K_D, kp=K_P):
    """Planar edge-feature kernel.

    Per chunk [128, cb] edges: vd2 = DMA in, v=a-b (DVE), d2 =
    max(|v|^2, tiny) (Pool); mid = sqrt (ACT), 1/d (DVE); back = RBF
    w_m^2 fused per plane into ACT Square(d + (-mu_m)), one
    exp(-w^2/sigma^2) over all 16 planes (ACT), SH products (DVE bf16),
    DMA out bf16. The body is modulo-software-pipelined (DMA-in leads
    by 3 chunks, v/d2 by 2) with the sqrts batched 3 chunks at a time
    so ACT pays only 2 activation-table loads (sqrt set vs
    square/exp set, 1283ns each) per 3 chunks.
    """
    nc = bacc.Bacc("TRN2", target_bir_lowering=False, debug=False,
                   num_devices=N_CORES)
    big_kind_i = "Internal" if reps else "ExternalInput"
    big_kind_o = "Internal" if reps else "ExternalOutput"
    ab_d = nc.dram_tensor("ab", [3, nch, 128, 6, cb], f32, kind=big_kind_i)
    od_d = nc.dram_tensor("od", [3, nch, 128, 25 * cb], bf16, kind=big_kind_o)
    mu_d = nc.dram_tensor("negmu", [128, NUM_RBF], f32, kind="ExternalInput")
    if reps:
        tok_d = nc.dram_tensor("tok", [128, NUM_RBF], f32,
                               kind="ExternalOutput")
    sub = mybir.AluOpType.subtract
    mul = mybir.AluOpType.mult
    add = mybir.AluOpType.add
    SQ = mybir.ActivationFunctionType.Square
    SQRT = mybir.ActivationFunctionType.Sqrt
    EXP = mybir.ActivationFunctionType.Exp
    DERF = mybir.ActivationFunctionType.Derivative_Erf
    HSQPI = 0.8862269254527580  # sqrt(pi)/2: Derivative_Erf = 2/sqrt(pi)*exp(-x^2)
    KDP = kd + kp
    nchunks = 3 * nch
    big = cb >= 600
    buf_ab = 3 if big else 4
    buf_v = 3 if big else 4
    buf_d2 = 3 if big else 4
    buf_di = 4 if big else 5
    buf_u = 1 if big else 2
    with tile.TileContext(nc) as tc:
        with (tc.tile_pool(name="io", bufs=2) as iop,
              tc.tile_pool(name="wk", bufs=2) as wkp):
            V, A, P = nc.vector, nc.scalar, nc.gpsimd
            negmu = iop.tile([128, NUM_RBF], f32, tag="negmu", bufs=1)
            nc.sync.dma_start(out=negmu[:], in_=mu_d.ap())
            if reps:
                tok_t = iop.tile([128, NUM_RBF], f32, tag="tok", bufs=1)
                V.tensor_tensor(out=tok_t[:], in0=negmu[:], in1=negmu[:],
                                op=add)
                nc.sync.dma_start(out=tok_d.ap(), in_=tok_t[:])
            # two rotating output buffers; the constant l0=1 plane (16) is
            # written once here and never touched again
            obufs = [iop.tile([128, 25, cb], bf16, tag="ob", name=f"ob{i}")
                     for i in range(2)]
            for ob in obufs:
                V.memset(ob[:, 16, :], 1.0)

            abufs = []
            if diag == "nodma":
                abufs = [iop.tile([128, 6, cb], f32, tag="ab", bufs=buf_ab,
                                  name=f"abf{i}") for i in range(buf_ab)]
                for abf in abufs:
                    V.memset(abf[:], 1.0)

            def dma_in(ci):
                if diag == "nodma":
                    return abufs[ci % buf_ab]
                t, n = divmod(ci, nch)
                ab = iop.tile([128, 6, cb], f32, tag="ab", bufs=buf_ab)
                nc.sync.dma_start(out=ab[:], in_=ab_d.ap()[t, n])
                return ab

            def vd2(ab, ci):
                # v = a-b (DVE), d2 = max(|v|^2, tiny) (Pool)
                v = wkp.tile([128, 3, cb], f32, tag="v", bufs=buf_v)
                d2 = wkp.tile([128, cb], f32, tag="d2", bufs=buf_d2)
                tq = wkp.tile([128, cb], f32, tag="tq", bufs=2)
                V.tensor_tensor(out=v[:], in0=ab[:, 0:3, :],
                                in1=ab[:, 3:6, :], op=sub)
                P.tensor_tensor(out=d2[:], in0=v[:, 0, :], in1=v[:, 0, :],
                                op=mul)
                P.tensor_tensor(out=tq[:], in0=v[:, 1, :], in1=v[:, 1, :],
                                op=mul)
                P.tensor_tensor(out=d2[:], in0=d2[:], in1=tq[:], op=add)
                P.tensor_tensor(out=tq[:], in0=v[:, 2, :], in1=v[:, 2, :],
                                op=mul)
                P.tensor_tensor(out=d2[:], in0=d2[:], in1=tq[:], op=add)
                P.tensor_scalar_max(d2[:], d2[:], 3e-16)
                return {"v": v, "d2": d2, "ci": ci}

            def mid(st):
                # d = sqrt(d2) (ACT, sqrt table set), 1/d (DVE)
                dp = wkp.tile([128, cb], f32, tag="dp", bufs=buf_di)
                inv = wkp.tile([128, cb], f32, tag="inv", bufs=buf_di)
                A.activation(dp[:], st["d2"][:], SQRT)
                V.reciprocal(inv[:], dp[:])
                st["dp"] = dp
                st["inv"] = inv

            def back(st):
                ci = st["ci"]
                t, n = divmod(ci, nch)
                v, dp, inv = st["v"], st["dp"], st["inv"]
                ob = obufs[ci % 2]
                u = wkp.tile([128, NUM_RBF, cb], bf16, tag="u", bufs=buf_u)
                # RBF: w_m^2 into the u staging tile (decouples RBF compute
                # from the 2-deep output rotation — writing ob directly
                # measured 30us/launch slower), then one exp into ob.
                # NOTE: a single-op-per-plane Gaussian via Derivative_Erf
                # (= 2/sqrt(pi) exp(-x^2)) measured correct on HW across the
                # whole range EXCEPT |x| ~< 1e-2..1e-8 where the pwp table
                # returns 0 instead of ~1.128 — fatal for edges with
                # d ~= mu_m (guaranteed by t2t self-edges), so it is NOT used.
                # negmu holds -mu/sigma: Square(d/sigma + (-mu_m/sigma)).
                for m in range(NUM_RBF):
                    A.activation(u[:, m, :], dp[:], SQ,
                                 bias=negmu[:, m:m + 1], scale=1.0 / SIGMA)
                A.activation(ob[:, 0:NUM_RBF, :], u[:], EXP, scale=-1.0)
                # SH block (bf16)
                r = wkp.tile([128, 3, cb], bf16, tag="r", bufs=1)
                rs = wkp.tile([128, 3, cb], bf16, tag="rs", bufs=1)
                z2 = wkp.tile([128, cb], bf16, tag="z2", bufs=1)
                m12 = wkp.tile([128, 2, cb], bf16, tag="m12", bufs=1)
                V.tensor_tensor(
                    out=r[:], in0=v[:],
                    in1=inv[:].rearrange("p (o c) -> p o c", o=1)
                        .to_broadcast([128, 3, cb]),
                    op=mul)
                V.tensor_scalar_mul(rs[:], r[:], S15)
                V.tensor_scalar_mul(ob[:, 17:20, :], r[:], S3)
                V.tensor_tensor(out=ob[:, 20:22, :], in0=rs[:, 0:2, :],
                                in1=r[:, 1:3, :], op=mul)
                V.tensor_tensor(out=ob[:, 23, :], in0=rs[:, 0, :],
                                in1=r[:, 2, :], op=mul)
                V.tensor_tensor(out=z2[:], in0=rs[:, 2, :], in1=r[:, 2, :],
                                op=mul)
                V.tensor_scalar(ob[:, 22, :], z2[:], 0.8660254037844386,
                                -0.5 * S5, op0=mul, op1=add)
                V.tensor_tensor(out=m12[:], in0=rs[:, 0:2, :],
                                in1=r[:, 0:2, :], op=mul)
                V.tensor_tensor(out=m12[:, 0, :], in0=m12[:, 0, :],
                                in1=m12[:, 1, :], op=sub)
                V.tensor_scalar(ob[:, 24, :], m12[:, 0, :], 0.5, None,
                                op0=mul)
                if diag != "nodma":
                    nc.sync.dma_start(out=od_d.ap()[t, n],
                                      in_=ob[:].rearrange("p f c -> p (f c)"))

            def body():
                # modulo schedule: DMA-in leads by 3 chunks, v/d2 by 2,
                # sqrt+recip batched per 3 chunks (2 ACT table loads per
                # batch), back at offset 0
                abs_ = {}
                sts = {}
                for ci in range(3):
                    abs_[ci] = dma_in(ci)
                for ci in range(2):
                    sts[ci] = vd2(abs_.pop(ci), ci)
                for c in range(nchunks):
                    if c % 3 == 0:
                        for ci in range(c, min(c + 3, nchunks)):
                            if ci not in sts:
                                sts[ci] = vd2(abs_.pop(ci), ci)
                            mid(sts[ci])
                    if c + 3 < nchunks:
                        abs_[c + 3] = dma_in(c + 3)
                    if c + 2 < nchunks and (c + 2) not in sts:
                        sts[c + 2] = vd2(abs_.pop(c + 2), c + 2)
                    back(sts.pop(c))

            if reps:
                with tc.For_i(0, reps):
                    body()
            else:
                body()
    nc.compile()
    return nc


def _seg_grids(trans_g, f_src, t_dst, W):
    """Host marshaling: place trans[f_src] rows + mask into padded per-core
    channel-planar segment grids [N_CORES, 128, 4, SEG_PER_PART*W]."""
    n = f_src.shape[0]
    order = np.argsort(t_dst, kind="stable")
    sd = t_dst[order]
    sf = f_src[order]
    starts = np.searchsorted(sd, np.arange(N_TFN))
    rank = np.arange(n) - starts[sd]
    core = sd // SEG_PER_CORE
    local = sd % SEG_PER_CORE
    p = local // SEG_PER_PART
    j = local % SEG_PER_PART
    FW = SEG_PER_PART * W
    grids = np.zeros((N_CORES, 128, 4, FW), np.float32)
    vals = trans_g[sf]  # [n, 3]
    pos = j * W + rank
    grids[core, p, 0, pos] = vals[:, 0]
    grids[core, p, 1, pos] = vals[:, 1]
    grids[core, p, 2, pos] = vals[:, 2]
    grids[core, p, 3, pos] = 1.0
    return grids


def _negmu_grid():
    return np.broadcast_to((-MU / SIGMA)[None, :],
                           (128, NUM_RBF)).astype(np.float32).copy()


def _marshal_b(trans, tfn_x, f_src, t_dst, t2t, t2f):
    """Planar per-core launch-B input grids [N_CORES, 3, NCH, 128, 6, CB].
    Pure index marshaling (gather + layout), no arithmetic."""
    ab = np.zeros((N_CORES, 3, NCH, 128, 6, CB), np.float32)
    pairs = ((trans[f_src], tfn_x[t_dst]),
             (tfn_x[t2t[0]], tfn_x[t2t[1]]),
             (tfn_x[t2f[0]], trans[t2f[1]]))
    tmp = np.zeros((PADE, 6), np.float32)
    for t, (arows, brows) in enumerate(pairs):
        for k in range(N_CORES):
            tmp[:EPC, 0:3] = arows[k * EPC:(k + 1) * EPC]
            tmp[:EPC, 3:6] = brows[k * EPC:(k + 1) * EPC]
            ab[k, t] = tmp.reshape(NCH, 128, CB, 6).transpose(0, 1, 3, 2)
    return ab


def _run_launch_a(trans, f_src, t_dst):
    cnts = np.bincount(t_dst, minlength=N_TFN)
    W = int(cnts.max())
    key = ("A", W)
    if key not in _cache:
        _cache[key] = _build_launch_a(W)
    ncA = _cache[key]
    grids = _seg_grids(trans, f_src, t_dst, W)
    in_maps = [{"grid": grids[k]} for k in range(N_CORES)]
    resA = bass_utils.run_bass_kernel_spmd(ncA, in_maps,
                                           core_ids=list(range(N_CORES)))
    tfn_x = np.zeros((SEG_PAD, 3), np.float32)
    for k in range(N_CORES):
        o = resA.results[k]["tfn"].reshape(128, 3, SEG_PER_PART)
        segs = (np.arange(128)[:, None] * SEG_PER_PART
                + np.arange(SEG_PER_PART)[None, :] + k * SEG_PER_CORE)
        tfn_x[segs.ravel()] = o.transpose(0, 2, 1).reshape(-1, 3)
    return tfn_x[:N_TFN]


def _run_launch_b(ab):
    if "B" not in _cache:
        _cache["B"] = _build_launch_b()
    ncB = _cache["B"]
    negmu = _negmu_grid()
    in_maps = [{"ab": ab[k], "negmu": negmu} for k in range(N_CORES)]
    resB = bass_utils.run_bass_kernel_spmd(ncB, in_maps,
                                           core_ids=list(range(N_CORES)))
    out = np.empty((3, E, NUM_RBF + 9), np.float32)
    for k in range(N_CORES):
        o = np.asarray(resB.results[k]["od"])
        o = o.reshape(3, NCH, 128, 25, CB).astype(np.float32)
        o = o.transpose(0, 1, 2, 4, 3).reshape(3, PADE, 25)
        out[:, k * EPC:(k + 1) * EPC, :] = o[:, :EPC, :]
    return out


def kernel(trans, frame2tfn_edge_index, tfn2tfn_edge_index,
           tfn2frame_edge_index, n_tfn):
    trans = np.asarray(trans, np.float32)
    f2t = np.asarray(frame2tfn_edge_index, np.int64)
    t2t = np.asarray(tfn2tfn_edge_index, np.int64)
    t2f = np.asarray(tfn2frame_edge_index, np.int64)
    f_src, t_dst = f2t[0], f2t[1]

    tfn_x = _run_launch_a(trans, f_src, t_dst)
    ab = _marshal_b(trans, tfn_x, f_src, t_dst, t2t, t2f)
    return _run_launch_b(ab)
